# revision 1
# baseline (speedup 1.0000x reference)
import os
import numpy as np

import concourse.bacc as bacc
import concourse.mybir as mybir
import concourse.tile as tile
from concourse.bass_utils import run_bass_kernel_spmd

# Problem constants (hardcoded per harness contract)
B, H, W, C = 32, 32, 32, 128
NUM, D0, D1 = 10, 60, 16
JK = D0 * D1            # 960
OO = NUM * JK           # 9600
P = H * W               # 1024 contraction dim of the dense kernel
N_CORES = 8
B_LOC = B // N_CORES    # 4 batches per core
BLK = 384               # dense-kernel column block (>=256 keeps fp32r at 1 cyc/row)
NBLK = OO // BLK        # 25
EPS = 1e-12

f32 = mybir.dt.float32
f32r = mybir.dt.float32r
AF = mybir.ActivationFunctionType
ALU = mybir.AluOpType


def build_nc():
    nc = bacc.Bacc("TRN2", debug=False)
    u_d = nc.dram_tensor("u", (B_LOC, H, W, C), f32, kind="ExternalInput").ap()
    wc_d = nc.dram_tensor("wc", (2, 2, C, C), f32, kind="ExternalInput").ap()
    km_d = nc.dram_tensor("km", (P, OO), f32, kind="ExternalInput").ap()
    eye_d = nc.dram_tensor("eye", (C, C), f32, kind="ExternalInput").ap()
    out_d = nc.dram_tensor("out", (B_LOC, NUM, JK), f32, kind="ExternalOutput").ap()

    with tile.TileContext(nc) as tc:
        with tc.tile_pool(name="persist", bufs=1) as pers:
            u_hat = pers.tile([128, B_LOC, OO], f32r)       # [n, b, o]
            uT = pers.tile([128, B_LOC, 8, 128], f32r)      # lhsT chunks [p, b, chunk, c]
            wct = pers.tile([128, 4, C], f32r)              # conv taps [ci, tap, co]
            eye = pers.tile([128, C], f32r)
            ones = pers.tile([128, 128], f32)
            crep0 = pers.tile([128, 128], f32r)             # uniform c = 0.1 (softmax of zeros)
            c_all = pers.tile([128, B_LOC, NUM], f32)
            z_all = pers.tile([128, B_LOC * NUM], f32)
            ss_all = pers.tile([128, B_LOC * NUM], f32)
            alpha = pers.tile([128, B_LOC * NUM], f32)
            blog = pers.tile([128, B_LOC * NUM], f32)
            eexp = pers.tile([128, B_LOC, NUM], f32)
            nmax = pers.tile([128, B_LOC], f32)
            sume = pers.tile([128, B_LOC], f32)
            rsum = pers.tile([128, B_LOC], f32)

            xpad = pers.tile([128, 33 * 33], f32r)
            zcol = pers.tile([128, 33], f32)

            nc.gpsimd.dma_start(wct[:], wc_d.rearrange("dh dw ci co -> ci (dh dw) co"))
            nc.gpsimd.dma_start(eye[:], eye_d)
            nc.vector.memset(ones[:], 1.0)
            nc.vector.memset(zcol[:], 0.0)
            nc.vector.tensor_scalar_mul(crep0[:], ones[:], 0.1)
            xpad_v = xpad[:].rearrange("p (h w) -> p h w", w=33)
            nc.vector.tensor_copy(xpad_v[:, :, 32], zcol[:])   # right pad col
            nc.vector.tensor_copy(xpad_v[:, 32, :], zcol[:])   # bottom pad row

            # ---------- Phase 1: 2x2 SAME conv, per batch ----------
            # out[co, s=h*32+w] = sum_taps Wtap.T @ xpad[:, (h+dh)*33 + (w+dw)]
            with tc.tile_pool(name="convp", bufs=2) as cvp, \
                 tc.tile_pool(name="psc", bufs=2, space="PSUM") as psc, \
                 tc.tile_pool(name="pst", bufs=2, space="PSUM") as pst:
                for b in range(B_LOC):
                    xin = cvp.tile([128, 8, 128], f32r, tag="xin")
                    nc.gpsimd.dma_start(
                        xin[:],
                        u_d[b].rearrange("h w c -> (h w) c").rearrange(
                            "(t sp) c -> sp t c", sp=128))
                    for t in range(8):
                        pt = pst.tile([128, 128], f32r, tag="pt")
                        nc.tensor.transpose(pt[:], xin[:, t, :], eye[:])
                        # pt[ch, sp] covers s = t*128 + sp -> rows h = t*4..t*4+4
                        src = pt[:].rearrange("p (a w) -> p a w", w=32)
                        dst = xpad_v[:, t * 4:(t + 1) * 4, 0:32]
                        if t % 2 == 0:
                            nc.vector.tensor_copy(dst, src)
                        else:
                            nc.scalar.copy(dst, src)
                    for hh in range(2):
                        pc = psc.tile([128, 512], f32, tag="pc")
                        for ti, (dh, dw) in enumerate(((0, 0), (0, 1), (1, 0), (1, 1))):
                            rhs = xpad_v[:, hh * 16 + dh: hh * 16 + dh + 16, dw:dw + 32]
                            nc.tensor.matmul(pc[:], wct[:, ti, :], rhs,
                                             start=(ti == 0), stop=(ti == 3))
                        # raw-reshape gather: uT[t][pp, c] = conv[a, 8q+t, pp], c = 4a+q
                        pcv = pc[:].rearrange("p (a q t) -> p a q t", q=4, t=8)
                        for t in range(8):
                            src = pcv[:, :, :, t]
                            dst = uT[:, b, t, hh * 64:(hh + 1) * 64].rearrange(
                                "p (a q) -> p a q", q=4)
                            if t % 2 == 0:
                                nc.vector.tensor_copy(dst, src)
                            else:
                                nc.scalar.copy(dst, src)

            # ---------- Phase 2: dense matmul u_hat = uT.T @ km ----------
            NPH = int(os.environ.get("KPHASES", "3"))
            with tc.tile_pool(name="kp", bufs=2) as kp, \
                 tc.tile_pool(name="psm", bufs=3, space="PSUM") as psm:
                kv = km_d.rearrange("(c p) o -> p c o", p=128)
                for blk in range(NBLK if NPH >= 2 else 0):
                    kt = kp.tile([128, 8, BLK], f32r, tag="kt")
                    nc.gpsimd.dma_start(kt[:], kv[:, :, blk * BLK:(blk + 1) * BLK])
                    for b in range(B_LOC):
                        pm = psm.tile([128, BLK], f32, tag="pm")
                        for ch in range(8):
                            nc.tensor.matmul(pm[:], uT[:, b, ch, :], kt[:, ch, :],
                                             start=(ch == 0), stop=(ch == 7))
                        dst = u_hat[:, b, blk * BLK:(blk + 1) * BLK]
                        if (blk * B_LOC + b) % 2 == 0:
                            nc.vector.tensor_copy(dst, pm[:])
                        else:
                            nc.scalar.copy(dst, pm[:])

            # ---------- Phase 3: dynamic routing (3 iterations) ----------
            with tc.tile_pool(name="rt", bufs=2) as rt, \
                 tc.tile_pool(name="psb", bufs=3, space="PSUM") as psb:
                KR = int(os.environ.get("KROUT", "5"))
                for it in range((3 if KR >= 5 else 1) if NPH >= 3 else 0):
                    for b in range(B_LOC):
                        for i in range(NUM):
                            if it == 0:
                                crep = crep0
                            else:
                                crep = rt.tile([128, 128], f32r, tag="crep")
                                nc.vector.tensor_scalar_mul(
                                    crep[:], ones[:], c_all[:, b, i:i + 1])
                            pbc = psb.tile([128, JK], f32, tag="pbc")
                            o0 = i * JK
                            nc.tensor.matmul(pbc[:, 0:512], crep[:],
                                             u_hat[:, b, o0:o0 + 512],
                                             start=True, stop=True)
                            nc.tensor.matmul(pbc[:, 512:JK], crep[:],
                                             u_hat[:, b, o0 + 512:o0 + JK],
                                             start=True, stop=True)
                            if it < 2 and KR >= 2:
                                un = b * NUM + i
                                scr = rt.tile([128, JK], f32, tag="scr")
                                scr2 = rt.tile([128, JK], f32, tag="scr2")
                                nc.vector.scalar_tensor_tensor(
                                    out=scr[:],
                                    in0=u_hat[:, b, o0:o0 + JK].bitcast(f32),
                                    scalar=1.0, in1=pbc[:],
                                    op0=ALU.mult, op1=ALU.mult,
                                    accum_out=z_all[:, un:un + 1])
                                if KR >= 3:
                                    nc.scalar.activation(
                                        scr2[:], pbc[:], AF.Square,
                                        accum_out=ss_all[:, un:un + 1])
                                else:
                                    nc.vector.memset(ss_all[:, un:un + 1], 1.0)
                            else:
                                ofin = rt.tile([1, JK], f32, tag="ofin")
                                if (b * NUM + i) % 2 == 0:
                                    nc.vector.tensor_copy(ofin[:], pbc[0:1, :])
                                else:
                                    nc.scalar.copy(ofin[:], pbc[0:1, :])
                                nc.sync.dma_start(out_d[b, i], ofin[:])
                    if it < 2 and KR >= 4:
                        # b_logits = z * rsqrt(max(ss, eps)); then softmax over capsules
                        nc.vector.tensor_scalar_max(ss_all[:], ss_all[:], EPS)
                        nc.scalar.activation(ss_all[:], ss_all[:], AF.Sqrt)
                        nc.vector.reciprocal(alpha[:], ss_all[:])
                        nc.vector.tensor_mul(blog[:], z_all[:], alpha[:])
                        blv = blog[:].rearrange("p (b i) -> p b i", i=NUM)
                        nc.vector.tensor_reduce(nmax[:], blv, axis=mybir.AxisListType.X,
                                                op=ALU.max, negate=True)
                        for b in range(B_LOC):
                            nc.scalar.activation(eexp[:, b, :], blv[:, b, :], AF.Exp,
                                                 bias=nmax[:, b:b + 1],
                                                 accum_out=sume[:, b:b + 1])
                        nc.vector.reciprocal(rsum[:], sume[:])
                        for b in range(B_LOC):
                            nc.vector.tensor_scalar_mul(
                                c_all[:, b, :], eexp[:, b, :], rsum[:, b:b + 1])
    nc.compile()
    return nc


_NC_CACHE = None


def _get_nc():
    global _NC_CACHE
    if _NC_CACHE is None:
        _NC_CACHE = build_nc()
    return _NC_CACHE


def kernel(u_vecs, W_conv, kernel):
    u_vecs = np.ascontiguousarray(np.asarray(u_vecs, dtype=np.float32))
    W_conv = np.ascontiguousarray(np.asarray(W_conv, dtype=np.float32))
    km = np.ascontiguousarray(np.asarray(kernel, dtype=np.float32))
    eye = np.eye(C, dtype=np.float32)
    nc = _get_nc()
    in_maps = [
        {"u": u_vecs[ci * B_LOC:(ci + 1) * B_LOC], "wc": W_conv, "km": km, "eye": eye}
        for ci in range(N_CORES)
    ]
    res = run_bass_kernel_spmd(nc, in_maps, core_ids=list(range(N_CORES)))
    out = np.concatenate([r["out"] for r in res.results], axis=0)
    return out.reshape(B, NUM, D0, D1).astype(np.float32)



# revision 4
# speedup vs baseline: 1.3753x; 1.3753x over previous
import numpy as np
import ml_dtypes

import concourse.bacc as bacc
import concourse.mybir as mybir
import concourse.tile as tile
from concourse.bass_utils import run_bass_kernel_spmd

# Problem constants (hardcoded per harness contract)
B, H, W, C = 32, 32, 32, 128
NUM, D0, D1 = 10, 60, 16
JK = D0 * D1            # 960
OO = NUM * JK           # 9600
P = H * W               # 1024 contraction dim of the dense kernel
N_CORES = 8
B_LOC = B // N_CORES    # 4 batches per core
HALF = JK // 2          # 480 (PSUM-bank-sized GEMM column block)
EPS = 1e-12

f32 = mybir.dt.float32
f32r = mybir.dt.float32r
bf16 = mybir.dt.bfloat16
AF = mybir.ActivationFunctionType
ALU = mybir.AluOpType


def build_nc():
    nc = bacc.Bacc("TRN2", debug=False)
    u_d = nc.dram_tensor("u", (B_LOC, H, W, C), f32, kind="ExternalInput").ap()
    wc_d = nc.dram_tensor("wc", (2, 2, C, C), f32, kind="ExternalInput").ap()
    # host pre-laid-out dense kernel: km[cap, p, c, col] = K[c*128+p, cap*960+col]
    km_d = nc.dram_tensor("km", (NUM, 128, 8, JK), bf16, kind="ExternalInput").ap()
    eye_d = nc.dram_tensor("eye", (C, C), f32, kind="ExternalInput").ap()
    out_d = nc.dram_tensor("out", (B_LOC, NUM, JK), f32, kind="ExternalOutput").ap()

    with tile.TileContext(nc) as tc:
        with tc.tile_pool(name="persist", bufs=1) as pers, \
             tc.tile_pool(name="kp", bufs=2) as kp:
            u_hat = pers.tile([128, B_LOC, OO], bf16)       # [n, b, o]
            uT = pers.tile([128, B_LOC, 8, 128], bf16)      # lhsT chunks [p, b, chunk, c]
            wct = pers.tile([128, 4, C], f32r)              # conv taps [ci, tap, co]
            eye = pers.tile([128, C], f32r)
            ones = pers.tile([128, 128], bf16)
            crep0 = pers.tile([128, 128], bf16)             # uniform c = 0.1 (softmax of zeros)
            crep_all = pers.tile([128, B_LOC, NUM, 128], bf16)
            c_all = pers.tile([128, B_LOC, NUM], f32)
            z_all = pers.tile([128, B_LOC, NUM], f32)
            ss_all = pers.tile([128, B_LOC, NUM], f32)
            alpha = pers.tile([128, B_LOC, NUM], f32)
            blog = pers.tile([128, B_LOC, NUM], f32)
            eexp = pers.tile([128, B_LOC, NUM], f32)
            nmax = pers.tile([128, B_LOC], f32)
            sume = pers.tile([128, B_LOC], f32)
            rsum = pers.tile([128, B_LOC], f32)

            xpad = pers.tile([128, 33 * 33], f32r)
            zcol = pers.tile([128, 33], f32)

            # prefetch the first dense-kernel capsule block during conv
            kt_first = kp.tile([128, 8, JK], bf16, tag="kt")
            nc.gpsimd.dma_start(kt_first[:], km_d[0])

            nc.gpsimd.dma_start(wct[:], wc_d.rearrange("dh dw ci co -> ci (dh dw) co"))
            nc.gpsimd.dma_start(eye[:], eye_d)
            nc.vector.memset(ones[:], 1.0)
            nc.vector.memset(zcol[:], 0.0)
            nc.vector.memset(crep0[:], 0.1)
            xpad_v = xpad[:].rearrange("p (h w) -> p h w", w=33)
            nc.vector.tensor_copy(xpad_v[:, :, 32], zcol[:])   # right pad col
            nc.vector.tensor_copy(xpad_v[:, 32, :], zcol[:])   # bottom pad row

            # ---------- Phase 1: 2x2 SAME conv, per batch ----------
            # out[co, s=h*32+w] = sum_taps Wtap.T @ xpad[:, (h+dh)*33 + (w+dw)]
            with tc.tile_pool(name="convp", bufs=2) as cvp, \
                 tc.tile_pool(name="psc", bufs=2, space="PSUM") as psc, \
                 tc.tile_pool(name="pst", bufs=2, space="PSUM") as pst:
                for b in range(B_LOC):
                    xin = cvp.tile([128, 8, 128], f32r, tag="xin")
                    nc.gpsimd.dma_start(
                        xin[:],
                        u_d[b].rearrange("h w c -> (h w) c").rearrange(
                            "(t sp) c -> sp t c", sp=128))
                    for t in range(8):
                        pt = pst.tile([128, 128], f32r, tag="pt")
                        nc.tensor.transpose(pt[:], xin[:, t, :], eye[:])
                        # pt[ch, sp] covers s = t*128 + sp -> rows h = t*4..t*4+4
                        src = pt[:].rearrange("p (a w) -> p a w", w=32)
                        dst = xpad_v[:, t * 4:(t + 1) * 4, 0:32]
                        if t % 2 == 0:
                            nc.vector.tensor_copy(dst, src)
                        else:
                            nc.scalar.copy(dst, src)
                    for hh in range(2):
                        pc = psc.tile([128, 512], f32, tag="pc")
                        for ti, (dh, dw) in enumerate(((0, 0), (0, 1), (1, 0), (1, 1))):
                            rhs = xpad_v[:, hh * 16 + dh: hh * 16 + dh + 16, dw:dw + 32]
                            nc.tensor.matmul(pc[:], wct[:, ti, :], rhs,
                                             start=(ti == 0), stop=(ti == 3))
                        # raw-reshape gather: uT[t][pp, c] = conv[a, 8q+t, pp], c = 4a+q
                        pcv = pc[:].rearrange("p (a q t) -> p a q t", q=4, t=8)
                        for t in range(8):
                            src = pcv[:, :, :, t]
                            dst = uT[:, b, t, hh * 64:(hh + 1) * 64].rearrange(
                                "p (a q) -> p a q", q=4)
                            if t % 2 == 0:
                                nc.vector.tensor_copy(dst, src)
                            else:
                                nc.scalar.copy(dst, src)

            def softmax_b(b):
                # b_logits = z * rsqrt(max(ss, eps)); softmax over capsules -> c_all
                nc.vector.tensor_scalar_max(ss_all[:, b], ss_all[:, b], EPS)
                nc.scalar.activation(ss_all[:, b], ss_all[:, b], AF.Sqrt)
                nc.vector.reciprocal(alpha[:, b], ss_all[:, b])
                nc.vector.tensor_mul(blog[:, b], z_all[:, b], alpha[:, b])
                nc.vector.tensor_reduce(nmax[:, b:b + 1], blog[:, b],
                                        axis=mybir.AxisListType.X,
                                        op=ALU.max, negate=True)
                nc.scalar.activation(eexp[:, b], blog[:, b], AF.Exp,
                                     bias=nmax[:, b:b + 1],
                                     accum_out=sume[:, b:b + 1])
                nc.vector.reciprocal(rsum[:, b:b + 1], sume[:, b:b + 1])
                nc.vector.tensor_scalar_mul(c_all[:, b], eexp[:, b],
                                            rsum[:, b:b + 1])

            # ---------- Phase 2: dense GEMM per capsule, iteration-0 routing woven in ----------
            with tc.tile_pool(name="rt", bufs=3) as rt, \
                 tc.tile_pool(name="psm", bufs=4, space="PSUM") as psm, \
                 tc.tile_pool(name="psb", bufs=2, space="PSUM") as psb:
                for cap in range(NUM):
                    if cap == 0:
                        kt = kt_first
                    else:
                        kt = kp.tile([128, 8, JK], bf16, tag="kt")
                        nc.gpsimd.dma_start(kt[:], km_d[cap])
                    o0 = cap * JK
                    for b in range(B_LOC):
                        for h in range(2):
                            pm = psm.tile([128, HALF], f32, tag="pm")
                            for ch in range(8):
                                nc.tensor.matmul(pm[:], uT[:, b, ch, :],
                                                 kt[:, ch, h * HALF:(h + 1) * HALF],
                                                 start=(ch == 0), stop=(ch == 7))
                            dst = u_hat[:, b, o0 + h * HALF:o0 + (h + 1) * HALF]
                            if (b + h) % 2 == 0:
                                nc.vector.tensor_copy(dst, pm[:])
                            else:
                                nc.scalar.copy(dst, pm[:])
                    # routing iteration 0 for this capsule (c uniform = 0.1)
                    for b in range(B_LOC):
                        pbc = psb.tile([128, JK], f32, tag="pbc")
                        nc.tensor.matmul(pbc[:, 0:512], crep0[:],
                                         u_hat[:, b, o0:o0 + 512],
                                         start=True, stop=True)
                        nc.tensor.matmul(pbc[:, 512:JK], crep0[:],
                                         u_hat[:, b, o0 + 512:o0 + JK],
                                         start=True, stop=True)
                        scr = rt.tile([128, JK], f32, tag="scr")
                        nc.vector.scalar_tensor_tensor(
                            out=scr[:],
                            in0=u_hat[:, b, o0:o0 + JK],
                            scalar=1.0, in1=pbc[:],
                            op0=ALU.mult, op1=ALU.mult,
                            accum_out=z_all[:, b, cap:cap + 1])
                        scr2 = rt.tile([128, JK], f32, tag="scr2")
                        nc.scalar.activation(
                            scr2[:], pbc[:], AF.Square,
                            accum_out=ss_all[:, b, cap:cap + 1])

                # ---------- Phase 3: routing iterations 1 (update) and 2 (final) ----------
                for b in range(B_LOC):
                    softmax_b(b)
                for it in (1, 2):
                    # build all replicated-c weight tiles first (frees the V queue
                    # for the STT accumulations that gate the next softmax)
                    for b in range(B_LOC):
                        for i in range(NUM):
                            nc.vector.tensor_scalar_mul(
                                crep_all[:, b, i], ones[:], c_all[:, b, i:i + 1])
                    for b in range(B_LOC):
                        for i in range(NUM):
                            o0 = i * JK
                            pbc = psb.tile([128, JK], f32, tag="pbc")
                            nc.tensor.matmul(pbc[:, 0:512], crep_all[:, b, i],
                                             u_hat[:, b, o0:o0 + 512],
                                             start=True, stop=True)
                            nc.tensor.matmul(pbc[:, 512:JK], crep_all[:, b, i],
                                             u_hat[:, b, o0 + 512:o0 + JK],
                                             start=True, stop=True)
                            if it == 1:
                                scr = rt.tile([128, JK], f32, tag="scr")
                                nc.vector.scalar_tensor_tensor(
                                    out=scr[:],
                                    in0=u_hat[:, b, o0:o0 + JK],
                                    scalar=1.0, in1=pbc[:],
                                    op0=ALU.mult, op1=ALU.mult,
                                    accum_out=z_all[:, b, i:i + 1])
                                scr2 = rt.tile([128, JK], f32, tag="scr2")
                                nc.scalar.activation(
                                    scr2[:], pbc[:], AF.Square,
                                    accum_out=ss_all[:, b, i:i + 1])
                            else:
                                ofin = rt.tile([1, JK], f32, tag="ofin")
                                if (b * NUM + i) % 2 == 0:
                                    nc.vector.tensor_copy(ofin[:], pbc[0:1, :])
                                    nc.sync.dma_start(out_d[b, i], ofin[:])
                                else:
                                    nc.scalar.copy(ofin[:], pbc[0:1, :])
                                    nc.gpsimd.dma_start(out_d[b, i], ofin[:])
                        if it == 1:
                            softmax_b(b)
    nc.compile()
    return nc


_NC_CACHE = None


def _get_nc():
    global _NC_CACHE
    if _NC_CACHE is None:
        _NC_CACHE = build_nc()
    return _NC_CACHE


def _prep_km(km):
    # km[cap, p, c, col] = K[c*128 + p, cap*960 + col], contiguous per partition
    kt = km.reshape(8, 128, NUM, JK).transpose(2, 1, 0, 3)
    return np.ascontiguousarray(kt).astype(ml_dtypes.bfloat16)


def kernel(u_vecs, W_conv, kernel):
    u_vecs = np.ascontiguousarray(np.asarray(u_vecs, dtype=np.float32))
    W_conv = np.ascontiguousarray(np.asarray(W_conv, dtype=np.float32))
    km = _prep_km(np.asarray(kernel, dtype=np.float32))
    eye = np.eye(C, dtype=np.float32)
    nc = _get_nc()
    in_maps = [
        {"u": u_vecs[ci * B_LOC:(ci + 1) * B_LOC], "wc": W_conv, "km": km, "eye": eye}
        for ci in range(N_CORES)
    ]
    res = run_bass_kernel_spmd(nc, in_maps, core_ids=list(range(N_CORES)))
    out = np.concatenate([r["out"] for r in res.results], axis=0)
    return out.reshape(B, NUM, D0, D1).astype(np.float32)


# revision 7
# speedup vs baseline: 1.5231x; 1.1075x over previous
import numpy as np
import ml_dtypes

import concourse.bacc as bacc
import concourse.mybir as mybir
import concourse.tile as tile
from concourse.bass_utils import run_bass_kernel_spmd

# Problem constants (hardcoded per harness contract)
B, H, W, C = 32, 32, 32, 128
NUM, D0, D1 = 10, 60, 16
JK = D0 * D1            # 960
OO = NUM * JK           # 9600
P = H * W               # 1024 contraction dim of the dense kernel
N_CORES = 8
B_LOC = B // N_CORES    # 4 batches per core
HALF = JK // 2          # 480 (PSUM-bank-sized GEMM column block)
EPS = 1e-12

f32 = mybir.dt.float32
f32r = mybir.dt.float32r
bf16 = mybir.dt.bfloat16
AF = mybir.ActivationFunctionType
ALU = mybir.AluOpType


def build_nc():
    nc = bacc.Bacc("TRN2", debug=False)
    # host pre-laid-out inputs (see _prep_* below)
    u_d = nc.dram_tensor("u", (B_LOC, 128, 8, C), f32, kind="ExternalInput").ap()
    wc_d = nc.dram_tensor("wc", (C, 4, C), f32, kind="ExternalInput").ap()
    km_d = nc.dram_tensor("km", (NUM, 128, 8, JK), bf16, kind="ExternalInput").ap()
    eye_d = nc.dram_tensor("eye", (C, C), f32, kind="ExternalInput").ap()
    out_d = nc.dram_tensor("out", (B_LOC, NUM, JK), f32, kind="ExternalOutput").ap()

    with tile.TileContext(nc) as tc:
        with tc.tile_pool(name="persist", bufs=1) as pers, \
             tc.tile_pool(name="kp", bufs=2) as kp:
            u_hat = pers.tile([128, B_LOC, OO], bf16)       # [n, b, o]
            uT = pers.tile([128, B_LOC, 8, 128], bf16)      # lhsT chunks [p, b, chunk, c]
            wct = pers.tile([128, 4, C], f32r)              # conv taps [ci, tap, co]
            eye = pers.tile([128, C], f32r)
            ones = pers.tile([128, 128], bf16)
            crep0 = pers.tile([128, 128], bf16)             # uniform c = 0.1 (softmax of zeros)
            crep_all = pers.tile([128, B_LOC, NUM, 128], bf16)
            crep2m = pers.tile([128, B_LOC, NUM, NUM], bf16)  # masked columns for final combine
            c_all = pers.tile([128, B_LOC, NUM], f32)
            z_all = pers.tile([128, B_LOC, NUM], f32)
            ss_all = pers.tile([128, B_LOC, NUM], f32)
            alpha = pers.tile([128, B_LOC, NUM], f32)
            blog = pers.tile([128, B_LOC, NUM], f32)
            eexp = pers.tile([128, B_LOC, NUM], f32)
            nmax = pers.tile([128, B_LOC], f32)
            sume = pers.tile([128, B_LOC], f32)
            rsum = pers.tile([128, B_LOC], f32)

            xpad = pers.tile([128, 33 * 33], f32r)
            zcol = pers.tile([128, 33], f32)

            nc.gpsimd.dma_start(wct[:], wc_d)
            nc.gpsimd.dma_start(eye[:], eye_d)
            # first dense-kernel capsule block prefetched during conv, on the
            # sync queue so it does not delay the conv input loads
            kt_first = kp.tile([128, 8, JK], bf16, tag="kt")
            nc.sync.dma_start(kt_first[:], km_d[0])

            nc.vector.memset(ones[:], 1.0)
            nc.vector.memset(zcol[:], 0.0)
            nc.vector.memset(crep0[:], 0.1)
            nc.vector.memset(crep2m[:], 0.0)
            xpad_v = xpad[:].rearrange("p (h w) -> p h w", w=33)
            nc.vector.tensor_copy(xpad_v[:, :, 32], zcol[:])   # right pad col
            nc.vector.tensor_copy(xpad_v[:, 32, :], zcol[:])   # bottom pad row

            # ---------- Phase 1: 2x2 SAME conv, per batch ----------
            # out[co, s=h*32+w] = sum_taps Wtap.T @ xpad[:, (h+dh)*33 + (w+dw)]
            with tc.tile_pool(name="convp", bufs=2) as cvp, \
                 tc.tile_pool(name="psc", bufs=2, space="PSUM") as psc, \
                 tc.tile_pool(name="pst", bufs=2, space="PSUM") as pst:
                for b in range(B_LOC):
                    xin = cvp.tile([128, 8, 128], f32r, tag="xin")
                    nc.gpsimd.dma_start(xin[:], u_d[b])
                    for t in range(8):
                        pt = pst.tile([128, 128], f32r, tag="pt")
                        nc.tensor.transpose(pt[:], xin[:, t, :], eye[:])
                        # pt[ch, sp] covers s = t*128 + sp -> rows h = t*4..t*4+4
                        src = pt[:].rearrange("p (a w) -> p a w", w=32)
                        dst = xpad_v[:, t * 4:(t + 1) * 4, 0:32]
                        if t % 2 == 0:
                            nc.vector.tensor_copy(dst, src)
                        else:
                            nc.scalar.copy(dst, src)
                    for hh in range(2):
                        pc = psc.tile([128, 512], f32, tag="pc")
                        for ti, (dh, dw) in enumerate(((0, 0), (0, 1), (1, 0), (1, 1))):
                            rhs = xpad_v[:, hh * 16 + dh: hh * 16 + dh + 16, dw:dw + 32]
                            nc.tensor.matmul(pc[:], wct[:, ti, :], rhs,
                                             start=(ti == 0), stop=(ti == 3))
                        # raw-reshape gather: uT[t][pp, c] = conv[a, 8q+t, pp], c = 4a+q
                        pcv = pc[:].rearrange("p (a q t) -> p a q t", q=4, t=8)
                        for t in range(8):
                            src = pcv[:, :, :, t]
                            dst = uT[:, b, t, hh * 64:(hh + 1) * 64].rearrange(
                                "p (a q) -> p a q", q=4)
                            if t % 2 == 0:
                                nc.vector.tensor_copy(dst, src)
                            else:
                                nc.scalar.copy(dst, src)

            def softmax_b(b):
                # b_logits = z * rsqrt(max(ss, eps)); softmax over capsules -> c_all
                nc.vector.tensor_scalar_max(ss_all[:, b], ss_all[:, b], EPS)
                nc.scalar.activation(ss_all[:, b], ss_all[:, b], AF.Sqrt)
                nc.vector.reciprocal(alpha[:, b], ss_all[:, b])
                nc.vector.tensor_mul(blog[:, b], z_all[:, b], alpha[:, b])
                nc.vector.tensor_reduce(nmax[:, b:b + 1], blog[:, b],
                                        axis=mybir.AxisListType.X,
                                        op=ALU.max, negate=True)
                nc.scalar.activation(eexp[:, b], blog[:, b], AF.Exp,
                                     bias=nmax[:, b:b + 1],
                                     accum_out=sume[:, b:b + 1])
                nc.vector.reciprocal(rsum[:, b:b + 1], sume[:, b:b + 1])
                nc.vector.tensor_scalar_mul(c_all[:, b], eexp[:, b],
                                            rsum[:, b:b + 1])

            with tc.tile_pool(name="rt", bufs=3) as rt, \
                 tc.tile_pool(name="rtg", bufs=2) as rtg, \
                 tc.tile_pool(name="psm", bufs=2, space="PSUM") as psm, \
                 tc.tile_pool(name="psb", bufs=3, space="PSUM") as psb:

                def zss_update(b, i, dst_i, on_gpsimd=False):
                    # o for (b, capsule i) -> PSUM broadcast; z/ss accumulations
                    o0 = i * JK
                    pbc = psb.tile([128, JK], f32, tag="pbc")
                    lhs = crep0[:] if dst_i is None else crep_all[:, b, i]
                    nc.tensor.matmul(pbc[:, 0:512], lhs,
                                     u_hat[:, b, o0:o0 + 512],
                                     start=True, stop=True)
                    nc.tensor.matmul(pbc[:, 512:JK], lhs,
                                     u_hat[:, b, o0 + 512:o0 + JK],
                                     start=True, stop=True)
                    if on_gpsimd:
                        scr = rtg.tile([128, JK], f32, tag="scrg")
                        eng = nc.gpsimd
                    else:
                        scr = rt.tile([128, JK], f32, tag="scr")
                        eng = nc.vector
                    eng.scalar_tensor_tensor(
                        out=scr[:],
                        in0=u_hat[:, b, o0:o0 + JK],
                        scalar=1.0, in1=pbc[:],
                        op0=ALU.mult, op1=ALU.mult,
                        accum_out=z_all[:, b, i:i + 1])
                    scr2 = rt.tile([128, JK], f32, tag="scr2")
                    nc.scalar.activation(
                        scr2[:], pbc[:], AF.Square,
                        accum_out=ss_all[:, b, i:i + 1])

                # ---- Phase 2: dense GEMM per capsule, iteration-0 routing woven in ----
                for cap in range(NUM):
                    if cap == 0:
                        kt = kt_first
                    else:
                        kt = kp.tile([128, 8, JK], bf16, tag="kt")
                        nc.gpsimd.dma_start(kt[:], km_d[cap])
                    o0 = cap * JK
                    for b in range(B_LOC):
                        for h in range(2):
                            pm = psm.tile([128, HALF], f32, tag="pm")
                            for ch in range(8):
                                nc.tensor.matmul(pm[:], uT[:, b, ch, :],
                                                 kt[:, ch, h * HALF:(h + 1) * HALF],
                                                 start=(ch == 0), stop=(ch == 7))
                            dst = u_hat[:, b, o0 + h * HALF:o0 + (h + 1) * HALF]
                            if (b + h) % 2 == 0:
                                nc.vector.tensor_copy(dst, pm[:])
                            else:
                                nc.scalar.copy(dst, pm[:])
                    # routing iteration 0 for this capsule (c uniform = 0.1)
                    for b in range(B_LOC):
                        zss_update(b, cap, None)

                # ---- Phase 3: routing iterations 1 (update) and 2 (final) ----
                for b in range(B_LOC):
                    softmax_b(b)
                for b in range(B_LOC):
                    for i in range(NUM):
                        nc.vector.tensor_scalar_mul(
                            crep_all[:, b, i], ones[:], c_all[:, b, i:i + 1])

                def it1_group(b):
                    for i in range(NUM):
                        zss_update(b, i, i)
                    softmax_b(b)
                    for i in range(NUM):
                        nc.vector.tensor_scalar_mul(
                            crep2m[:, b, i, i:i + 1], ones[:, 0:1],
                            c_all[:, b, i:i + 1])

                def it2_group(b):
                    # all 10 capsules' o accumulated into one [10, 960] tile via
                    # masked lhsT columns, then one copy + one DMA per batch
                    pf = psb.tile([128, JK], f32, tag="pbc")
                    for i in range(NUM):
                        nc.tensor.matmul(pf[0:NUM, 0:512], crep2m[:, b, i],
                                         u_hat[:, b, i * JK:i * JK + 512],
                                         start=(i == 0), stop=(i == NUM - 1))
                    for i in range(NUM):
                        nc.tensor.matmul(pf[0:NUM, 512:JK], crep2m[:, b, i],
                                         u_hat[:, b, i * JK + 512:(i + 1) * JK],
                                         start=(i == 0), stop=(i == NUM - 1))
                    ofin = rt.tile([NUM, JK], f32, tag="ofin")
                    if b % 2 == 0:
                        nc.vector.tensor_copy(ofin[:], pf[0:NUM, :])
                    else:
                        nc.scalar.copy(ofin[:], pf[0:NUM, :])
                    nc.sync.dma_start(out_d[b], ofin[:])

                it1_group(0)
                it1_group(1)
                it2_group(0)
                it1_group(2)
                it2_group(1)
                it1_group(3)
                it2_group(2)
                it2_group(3)
    nc.compile()
    return nc


_NC_CACHE = None


def _get_nc():
    global _NC_CACHE
    if _NC_CACHE is None:
        _NC_CACHE = build_nc()
    return _NC_CACHE


def _prep_km(km):
    # km[cap, p, c, col] = K[c*128 + p, cap*960 + col], contiguous per partition
    kt = km.reshape(8, 128, NUM, JK).transpose(2, 1, 0, 3)
    return np.ascontiguousarray(kt).astype(ml_dtypes.bfloat16)


def _prep_u(u):
    # [B, (t sp), c] -> [B, sp, t, c] so each partition's row is contiguous
    return np.ascontiguousarray(
        u.reshape(B, 8, 128, C).transpose(0, 2, 1, 3))


def _prep_wc(wc):
    # [dh, dw, ci, co] -> [ci, (dh dw), co]
    return np.ascontiguousarray(wc.transpose(2, 0, 1, 3).reshape(C, 4, C))


def kernel(u_vecs, W_conv, kernel):
    u_vecs = _prep_u(np.ascontiguousarray(np.asarray(u_vecs, dtype=np.float32)))
    wc = _prep_wc(np.asarray(W_conv, dtype=np.float32))
    km = _prep_km(np.asarray(kernel, dtype=np.float32))
    eye = np.eye(C, dtype=np.float32)
    nc = _get_nc()
    in_maps = [
        {"u": u_vecs[ci * B_LOC:(ci + 1) * B_LOC], "wc": wc, "km": km, "eye": eye}
        for ci in range(N_CORES)
    ]
    res = run_bass_kernel_spmd(nc, in_maps, core_ids=list(range(N_CORES)))
    out = np.concatenate([r["out"] for r in res.results], axis=0)
    return out.reshape(B, NUM, D0, D1).astype(np.float32)
